# revision 23
# baseline (speedup 1.0000x reference)
"""Bass/Tile TRN2 kernel for a 3x3 locally-connected (unshared-weight) layer.

Computation (per batch row b, grid unit h, hw = 256*256):
    y[b,h] = sigmoid( sum_o x[b, nbr_idx[o,h]] * (valid[o,h] ? weights[o,h] : 0) )
    y[b,h] = sigmoid(0) = 0.5 where ~fault_mask[h] (mask applied pre-sigmoid)

Strategy: the gather is a fixed 3x3 stencil (verified on host at call time).
The grid (256x256) is tiled into 8x16 output patches (128 outputs = full PE
width).  A patch's 9-point stencil inputs form its 10x18 hull (180 grid
cells); with x transposed to (cell, batch), each patch is TWO matmuls:
    psum[128 out, 256 batch]  = lhsT_A[128 hull-rows, 128].T @ xh_A[128, 256]
    psum                     += lhsT_B[ 52 hull-rows, 128].T @ xh_B[ 52, 256]
where the lhsT blocks hold the (mostly zero) scattered effective weights.
ScalarE applies sigmoid per 4-patch PSUM quad, writing bf16; faulted units
are fixed up on the host (sigmoid(0) = 0.5).

Weight-block traffic is 360 B/output (vs 128-row dy-band blocks' 780 B) and
hull packing duplicates x by 180/128 = 1.41x; both streams plus the bf16
output total ~13 MB/core vs ~20 MB for the banded formulation -- this kernel
is HBM-DMA-bound, so bytes are the currency.

Sharding: gy is split 8 ways (32 grid rows = 4 patch-rows of 16 patches per
core); batch (256) rides along the matmul free dimension.  Every core runs
an identical program; grid-boundary effects are encoded in host-built
zero-padded hulls / zero weight blocks.  All inputs are SBUF-resident
(~100 KB/partition) and DMA'd up-front, balanced ~3.1 MB per dynamic DMA
queue in consumption order.  Hard-won scheduling rules baked in below:
every DMA spans all 128 partitions (bandwidth scales with partition
coverage, so 52-row B-halves pack two groups per 128-row tile); matmuls
run in same-shape runs so LDWEIGHTS pipelines (~131 ns/MM vs ~390);
the Scalar/ACT engine issues few input DMAs and no stores (a dma_start
blocked on ring capacity stalls every later ACTIVATE in its FIFO); one
4-bank ACT per group keeps the serial sigmoid chain off the critical
path; dummy matmuls after the first three groups hold the PE's HAM
clock-gate at 8/8 through the input-starved ramp.
"""

import numpy as np
import ml_dtypes

BATCH = 256
W = 256               # grid width/height
HW = W * W
N_CONN = 9
PA, PB = 8, 16        # patch shape (gy x gx) -> M = 128 outputs
HA, HB = PA + 2, PB + 2   # hull shape 10 x 18 -> K = 180, split 128 + 52
KSPLIT = 128
K2 = HA * HB - KSPLIT     # 52
K2P = 64              # B-half padded stride: two groups pack one 128-row tile
NPAIR_G = 4           # group pairs per core
NCORES = 8
NPY, NPX = W // PA, W // PB      # 32 x 16 patch grid
PRPC = NPY // NCORES             # 4 patch-rows per core
NGRP = PRPC * 2                  # 8 half-row DMA groups (8 patches each)
GP = NPX // 2                    # patches per group
NPATCH = PRPC * NPX              # 64 patches per core

_BF16 = ml_dtypes.bfloat16


def _build_patch_weights(weights, nbr_idx, valid):
    """Scatter effective weights into per-patch lhsT blocks.

    Returns W4 float32 (NPY*NPX, HA*HB, 128): for patch P, W4[P, k, m] is the
    weight of the connection feeding output m (= oy*16+ox) from hull cell k
    (= hy*18+hx, hull origin one cell up-left of the patch).  Raises
    ValueError if some valid (o,h) connection is not coverable.
    """
    h = np.arange(HW, dtype=np.int64)
    gy, gx = h // W, h % W
    PY, PX = gy // PA, gx // PB
    P = PY * NPX + PX
    m = (gy % PA) * PB + (gx % PB)

    g = nbr_idx.astype(np.int64)
    vm = valid.astype(bool)
    w_eff = np.where(vm, weights.astype(np.float32), 0.0)

    hy = g // W - (PA * PY - 1)
    hx = g % W - (PB * PX - 1)
    inh = (hy >= 0) & (hy < HA) & (hx >= 0) & (hx < HB)
    if not np.all(inh | ~vm):
        raise ValueError(
            "nbr_idx is not coverable by the patch-stencil kernel "
            f"({np.count_nonzero(vm & ~inh)} uncovered connections)"
        )
    k = hy * HB + hx
    mask = vm & inh
    Pb = np.broadcast_to(P, g.shape)
    mb = np.broadcast_to(m, g.shape)
    W4 = np.zeros((NPY * NPX, HA * HB, 128), dtype=np.float32)
    np.add.at(W4, (Pb[mask], k[mask], mb[mask]), w_eff[mask])
    return W4


def _build_program():
    import concourse.bacc as bacc
    import concourse.mybir as mybir
    from concourse import tile
    from concourse._compat import axon_active

    nc = bacc.Bacc(
        "TRN2",
        target_bir_lowering=False,
        debug=not axon_active(),
        num_devices=NCORES,
    )
    f32 = mybir.dt.float32
    bf16 = mybir.dt.bfloat16

    xh1_d = nc.dram_tensor("xh1", [NGRP, KSPLIT, GP * 256], bf16, kind="ExternalInput")
    xh2_d = nc.dram_tensor("xh2", [NPAIR_G, 128, GP * 256], bf16, kind="ExternalInput")
    wt1_d = nc.dram_tensor("wt1", [NGRP, KSPLIT, GP * 128], bf16, kind="ExternalInput")
    wt2_d = nc.dram_tensor("wt2", [NPAIR_G, 128, GP * 128], bf16, kind="ExternalInput")
    yt_d = nc.dram_tensor("yt", [128, NPATCH * 256], bf16, kind="ExternalOutput")

    with tile.TileContext(nc) as tc:
        with (
            tc.tile_pool(name="xh", bufs=1) as xh_pool,
            tc.tile_pool(name="wt", bufs=1) as wt_pool,
            tc.tile_pool(name="const", bufs=1) as const_pool,
            tc.tile_pool(name="out", bufs=6) as out_pool,
            tc.tile_pool(name="psum", bufs=2, space="PSUM") as psum_pool,
        ):
            # All inputs SBUF-resident; every DMA issued up-front in
            # consumption order and spanning all 128 partitions -- DMA
            # bandwidth scales with partitions-used/128, so 52-row B-half
            # transfers are packed two groups per 128-row tile (each half
            # padded to a 64-row stride; matmuls read rows [64h, 64h+52)).
            # Traffic is balanced across the three dynamic DMA queues
            # (sync / scalar HWDGE + gpsimd SWDGE).
            xh1_sb, wt1_sb, xh2_sb, wt2_sb = [], [], [], []
            for g in range(NGRP):
                xh1_sb.append(xh_pool.tile([KSPLIT, GP * 256], bf16, tag=f"xh1_{g}", name=f"xh1_{g}"))
                wt1_sb.append(wt_pool.tile([KSPLIT, GP * 128], bf16, tag=f"wt1_{g}", name=f"wt1_{g}"))
            for p in range(NPAIR_G):
                xh2_sb.append(xh_pool.tile([128, GP * 256], bf16, tag=f"xh2_{p}", name=f"xh2_{p}"))
                wt2_sb.append(wt_pool.tile([128, GP * 128], bf16, tag=f"wt2_{p}", name=f"wt2_{p}"))
            # Queue discipline: HWDGE rings stall the ISSUING ENGINE once
            # ~4 transfers are outstanding, so the Scalar/ACT engine gets
            # few input DMAs (all preceding every ACTIVATE) and no stores.
            # The SWDGE (gpsimd) ring has ~3.5 us to first byte, so it only
            # carries tensors needed later; input bytes ~3.1 MB per ring,
            # in consumption order.
            for t, idx in (("xh1", 0), ("wt1", 0), ("wt2", 0), ("xh1", 2),
                           ("wt1", 3), ("xh1", 4), ("wt1", 5)):
                src_d = {"xh1": xh1_d, "wt1": wt1_d, "wt2": wt2_d}[t]
                dst = {"xh1": xh1_sb, "wt1": wt1_sb, "wt2": wt2_sb}[t]
                nc.sync.dma_start(out=dst[idx][:, :], in_=src_d[idx])
            H2 = GP * 128  # split the last-arriving input so the final
            nc.sync.dma_start(out=xh1_sb[7][:, 0:H2], in_=xh1_d[7][:, 0:H2])
            nc.sync.dma_start(out=xh1_sb[7][:, H2:], in_=xh1_d[7][:, H2:])
            for t, idx in (("xh2", 0), ("xh1", 1), ("xh2", 1),
                           ("xh1", 3), ("xh2", 2), ("xh2", 3)):
                src_d = {"xh1": xh1_d, "xh2": xh2_d}[t]
                dst = {"xh1": xh1_sb, "xh2": xh2_sb}[t]
                nc.scalar.dma_start(out=dst[idx][:, :], in_=src_d[idx])
            for t, idx in (("wt1", 1), ("wt1", 2), ("wt2", 1), ("wt1", 4),
                           ("wt2", 2), ("xh1", 5), ("xh1", 6), ("wt2", 3),
                           ("wt1", 6), ("wt1", 7)):
                src_d = {"xh1": xh1_d, "wt1": wt1_d, "wt2": wt2_d}[t]
                dst = {"xh1": xh1_sb, "wt1": wt1_sb, "wt2": wt2_sb}[t]
                nc.gpsimd.dma_start(out=dst[idx][:, :], in_=src_d[idx])

            # PE pre-warm: dummy matmuls on zeroed SBUF while the first input
            # DMAs are in flight, so the HAM clock-gate opens (1.2 -> 2.4 GHz)
            # before the real matmul stream begins.
            warm_sb = const_pool.tile([128, 640], bf16, tag="warm")
            nc.vector.memset(warm_sb[:, :], 0.0)
            warm_ps = psum_pool.tile([128, 2048], f32, tag="pA", name="warm_ps")
            for _ in range(10):
                nc.tensor.matmul(
                    warm_ps[:, 0:512],
                    warm_sb[:, 0:128],
                    warm_sb[:, 128:640],
                    start=True,
                    stop=True,
                )

            # Per group: eight K=90 A-matmuls back-to-back, then eight K=90
            # B-matmuls.  Identical LDWEIGHTS shapes keep the PE's
            # background-weight-buffer pull-ahead alive (~131 ns/MM);
            # alternating shapes serialize it (~390 ns/MM).
            store_eng = [nc.gpsimd, nc.gpsimd, nc.gpsimd, nc.sync,
                         nc.gpsimd, nc.sync, nc.gpsimd, nc.sync]
            for g in range(NGRP):
                b0 = (g % 2) * K2P  # B-half base partition within the pair tile
                ps = psum_pool.tile([128, 2048], f32, tag="pA", name=f"ps_{g}")
                for px in range(GP):
                    co = px * 256
                    # start=True on each 512-wide bank's first MM clears that
                    # bank's has_written bits; later MMs (start=False)
                    # overwrite fresh cells and accumulate onto written ones.
                    nc.tensor.matmul(
                        ps[:, co : co + 256],
                        wt1_sb[g][:, px * 128 : (px + 1) * 128],
                        xh1_sb[g][:, px * 256 : (px + 1) * 256],
                        start=(px % 2 == 0),
                        stop=False,
                        skip_group_check=True,
                    )
                ot = out_pool.tile([128, 2048], bf16)
                for px in range(GP):
                    co = px * 256
                    nc.tensor.matmul(
                        ps[:, co : co + 256],
                        wt2_sb[g // 2][b0 : b0 + K2, px * 128 : (px + 1) * 128],
                        xh2_sb[g // 2][b0 : b0 + K2, px * 256 : (px + 1) * 256],
                        start=False,
                        stop=(px % 2 == 1),
                        skip_group_check=True,
                    )
                if g == NGRP - 1:
                    # final group: per-bank sigmoid + quarter store so the
                    # post-stream tail is one small ACT + one small store
                    for bk in range(4):
                        nc.scalar.activation(
                            ot[:, bk * 512 : (bk + 1) * 512],
                            ps[:, bk * 512 : (bk + 1) * 512],
                            mybir.ActivationFunctionType.Sigmoid,
                            bias=0.0,
                            scale=1.0,
                        )
                        store_eng[g].dma_start(
                            out=yt_d[:, g * 2048 + bk * 512 : g * 2048 + (bk + 1) * 512],
                            in_=ot[:, bk * 512 : (bk + 1) * 512],
                        )
                else:
                    # one 4-bank sigmoid per group: the serial ACT chain on
                    # the Scalar engine paces the back half, and fewer,
                    # larger ACTs amortize the per-instruction cost
                    nc.scalar.activation(
                        ot[:, 0:2048],
                        ps[:, 0:2048],
                        mybir.ActivationFunctionType.Sigmoid,
                        bias=0.0,
                        scale=1.0,
                    )
                    store_eng[g].dma_start(
                        out=yt_d[:, g * 2048 : (g + 1) * 2048],
                        in_=ot[:, :],
                    )
                if g < 3:
                    # HAM-warmkeeping filler: dummy matmuls occupy the PE
                    # through the early input-starved gaps so the clock gate
                    # stays at 8/8.  Their garbage lands in a PSUM slot that
                    # a later group's start=True matmul clears anyway.
                    for _ in range(8):
                        nc.tensor.matmul(
                            warm_ps[:, 0:512],
                            warm_sb[:, 0:128],
                            warm_sb[:, 128:640],
                            start=True,
                            stop=True,
                        )
    nc.compile()
    return nc


TRACE = False          # set by test harness to capture an NTFF profile
LAST_RESULTS = None    # BassKernelResults of the most recent run
_NC_CACHE = None       # compiled program, reused across calls


def kernel(x, weights, nbr_idx, valid, fault_mask):
    global LAST_RESULTS
    from concourse.bass_utils import run_bass_kernel_spmd

    x = np.asarray(x)
    out_dtype = x.dtype

    W4 = _build_patch_weights(
        np.asarray(weights), np.asarray(nbr_idx), np.asarray(valid)
    ).astype(_BF16)

    # x -> zero-padded (258, 258, B) grid, bf16
    xtp = np.zeros((W + 2, W + 2, BATCH), dtype=_BF16)
    xtp[1 : W + 1, 1 : W + 1] = (
        np.ascontiguousarray(x.T).astype(_BF16).reshape(W, W, BATCH)
    )
    # all patch hulls: (NPY, NPX, HA*HB, B)
    sl = np.lib.stride_tricks.sliding_window_view(xtp, (HA, HB), axis=(0, 1))
    hulls = (
        sl[::PA, ::PB]                      # (NPY, NPX, B, HA, HB)
        .transpose(0, 1, 3, 4, 2)
        .reshape(NPY, NPX, HA * HB, BATCH)
    )

    W4 = W4.reshape(NPY, NPX, HA * HB, 128)
    in_maps = []
    for c in range(NCORES):
        hc = hulls[c * PRPC : (c + 1) * PRPC]   # (PRPC, NPX, 180, B)
        wc = W4[c * PRPC : (c + 1) * PRPC]      # (PRPC, NPX, 180, 128)
        # half-row groups of GP=8 patches: [NGRP, 180, GP, .]
        hg = hc.reshape(NGRP, GP, HA * HB, BATCH).transpose(0, 2, 1, 3)
        wg = wc.reshape(NGRP, GP, HA * HB, 128).transpose(0, 2, 1, 3)
        hb = np.zeros((NPAIR_G, 2, K2P, GP, BATCH), dtype=hg.dtype)
        hb[:, :, :K2] = hg[:, KSPLIT:].reshape(NPAIR_G, 2, K2, GP, BATCH)
        wb = np.zeros((NPAIR_G, 2, K2P, GP, 128), dtype=wg.dtype)
        wb[:, :, :K2] = wg[:, KSPLIT:].reshape(NPAIR_G, 2, K2, GP, 128)
        in_maps.append(
            {
                "xh1": np.ascontiguousarray(hg[:, :KSPLIT]).reshape(
                    NGRP, KSPLIT, GP * 256
                ),
                "xh2": np.ascontiguousarray(hb).reshape(NPAIR_G, 128, GP * 256),
                "wt1": np.ascontiguousarray(wg[:, :KSPLIT]).reshape(
                    NGRP, KSPLIT, GP * 128
                ),
                "wt2": np.ascontiguousarray(wb).reshape(NPAIR_G, 128, GP * 128),
            }
        )

    global _NC_CACHE
    if _NC_CACHE is None:
        _NC_CACHE = _build_program()
    nc = _NC_CACHE
    res = run_bass_kernel_spmd(
        nc, in_maps, core_ids=list(range(NCORES)), trace=TRACE
    )
    LAST_RESULTS = res

    # unshard: per-core yt is [m=oy*16+ox, NPATCH*256] with patches in
    # (patch-row-major, quad) order -> (B, HW)
    parts = []
    for c, r in enumerate(res.results):
        yt = np.asarray(r["yt"]).reshape(PA, PB, PRPC, NPX, BATCH)
        # [oy, ox, pyl, px, b] -> [b, pyl, oy, px, ox]
        parts.append(
            yt.transpose(4, 2, 0, 3, 1).reshape(BATCH, PRPC * PA, W)
        )
    y = np.concatenate(parts, axis=1).reshape(BATCH, HW).astype(out_dtype, copy=False)
    # faulted units: reference computes sigmoid(where(fault, y, 0)) -> 0.5
    fault = np.asarray(fault_mask).astype(bool)
    y[:, ~fault] = np.float32(0.5)
    return y
